# revision 93
# baseline (speedup 1.0000x reference)
"""Multi-head dot-product attention (RoPE, causal) on 8 NeuronCores.

Sharding: data-parallel over batch (2) x tensor-parallel over heads (16 -> 4
per core). Each core projects q/k/v for its 4 heads, runs causal attention,
and computes a partial output projection; the host sums the 4 partials per
batch element.

v3 design notes (on top of the v2 bf16 baseline; 389us -> ~349us):
- The PE stream (~300us of bf16 matmul at 2.4GHz) is the roofline; v3 cuts
  PE work where numerically free and removes most PE idle by keeping every
  PE-gating producer (exp on ACT, tri/rope/recip on DVE/GpSimd) off each
  other's in-order queues:
  * ropes run mostly in bf16 (tables, intermediates): DVE 2X_1PORT gives 2x
    per op. In phase 1 the idle ACT evicts k twice (straight + partition-
    swapped halves) so all three rope ops are same-start-partition bf16; in
    phase 2 the two swap-muls read the q PSUM directly (cross-partition
    reads are only legal from PSUM) and only the straight mul + add are bf16.
  * the causal mask is no longer an f32 PSUM add before exp: exp runs
    unmasked and a [128,128] bf16 0/1-triangle multiply on the otherwise
    idle GpSimd zeroes the boundary tile of eG post-exp.
  * softmax reciprocal uses custom-DVE reciprocal_approx_fast (~5x cheaper,
    18 significant bits).
  * the denominator's non-diagonal matmuls run as fp8e4 DoubleRow pairs
    (2 key-slices contracted per instruction): a DVE tensor_scalar copies
    eG to a 1/16-scaled fp8 shadow (dodging the TRN fp8e4 240-max -> Inf;
    ones8 holds 16.0 to compensate). den is a sum of positives so the fp8
    noise averages out (<1e-4 on the final output). ~10us less PE.
  * the output projection is woven into the attention loop as PE filler:
    each t-block's 8 [128,2,512] out-chains run inside the NEXT t-block's
    attention (no ACT/DVE dependencies -> they cover the exp-gated den
    stalls), sharing the score PSUM pool. Evictions on DVE and stores on
    the sync queue ONLY: an eviction copy or a store-dispatch on the ACT
    queue delays an exp that gates a den chain (measured +1us/block).
  * qT holds only the live t-block ([128,512] per head) to make room for
    the fp8 eG shadow in SBUF.
- Startup: `ones` comes from a GpSimd memset (no DMA wait) so warmup matmuls
  start right at engine bringup (~7us); the first ~35us are DMA-supply-bound,
  so wk/wv interleave on one queue in ascending >=4KB-descriptor chunks
  (arrival order = et consumption order), xkv is chunked so chains start on
  the first half, and wq/wo/xq are deferred out of the critical window.
- Phase 1 ends staggered (last K chain's eviction reads overlap the last V
  chains) so phase 2's first Q chain isn't gated by the PSUM pool drain.
- PSUM: everything round-robins tag "sp" (3 x [128,2,512] = 6 banks) +
  den (1) + AV (1) = 8 banks exactly.
"""

import numpy as np

B, S, E, N, D = 2, 2048, 2048, 16, 128
HL = 4           # local heads per core (8 cores = 2 batch x 4 head groups)
ND = HL * D      # 512
NT = S // 128    # 16 row tiles
NB = S // 512    # 4 row blocks
NE = E // 128    # 16 contraction tiles

_NC_CACHE = {}


def _build_module():
    import concourse.bass as bass
    import concourse.mybir as mybir
    import concourse.tile as tile
    from concourse import bacc

    f32 = mybir.dt.float32
    bf16 = mybir.dt.bfloat16
    f8 = mybir.dt.float8e4
    DR = mybir.MatmulPerfMode.DoubleRow
    Exp = mybir.ActivationFunctionType.Exp

    nc = bacc.Bacc("TRN2", target_bir_lowering=False, debug=False, num_devices=8)

    # Host-packed layouts (flat contiguous DMAs):
    xq_d = nc.dram_tensor("xq_p", [NB, 128, NE, 512], bf16, kind="ExternalInput").ap()
    xkv_d = nc.dram_tensor("xkv_p", [NB, 128, NE, 512], bf16, kind="ExternalInput").ap()
    wq_d = nc.dram_tensor("wq_p", [128, NE, ND], bf16, kind="ExternalInput").ap()
    wk_d = nc.dram_tensor("wk_p", [128, NE, ND], bf16, kind="ExternalInput").ap()
    wv_d = nc.dram_tensor("wv_p", [128, NE, ND], bf16, kind="ExternalInput").ap()
    wo_d = nc.dram_tensor("wo_p", [128, HL, E], bf16, kind="ExternalInput").ap()
    csd_d = nc.dram_tensor("csd", [128, S], bf16, kind="ExternalInput").ap()
    sns_d = nc.dram_tensor("sns", [128, S], bf16, kind="ExternalInput").ap()
    tri_d = nc.dram_tensor("tri", [128, 128], bf16, kind="ExternalInput").ap()
    out_d = nc.dram_tensor("out", [NT, 128, E], bf16, kind="ExternalOutput").ap()

    with tile.TileContext(nc) as tc:
        with tc.tile_pool(name="const", bufs=1) as cpool, \
             tc.tile_pool(name="wqo", bufs=1) as wqo_pool, \
             tc.tile_pool(name="xq", bufs=2) as xq_pool, \
             tc.tile_pool(name="persist", bufs=1) as pers_pool:
            csd = cpool.tile([128, S], bf16, tag="csd")
            sns = cpool.tile([128, S], bf16, tag="sns")
            tri = cpool.tile([128, 128], bf16, tag="tri")
            ones = cpool.tile([128, 128], bf16, tag="ones")
            ones8 = cpool.tile([128, 2, 128], f8, tag="ones8")
            wq = wqo_pool.tile([128, NE, ND], bf16, tag="wq")
            wo = wqo_pool.tile([128, HL, E], bf16, tag="wo")
            kT = [pers_pool.tile([128, S], bf16, tag=f"kT{h}", name=f"kT{h}")
                  for h in range(HL)]
            vG = [pers_pool.tile([128, 4, ND], bf16, tag=f"vG{g}", name=f"vG{g}")
                  for g in range(NB)]
            uT = [pers_pool.tile([128, S], bf16, tag=f"uT{h}", name=f"uT{h}")
                  for h in range(HL)]
            xq_tiles = {}

            # no DMA dependency: the den stationary + warmup fodder is a
            # memset so the PE can start chewing at ~1us
            nc.gpsimd.memset(ones[:], 1.0)
            # 16.0: the fp8 eG shadow is scaled by 1/16 to stay below the
            # TRN-fp8e4 240 max (raw exp values reach ~e^5.5 = 245 -> Inf)
            nc.gpsimd.memset(ones8[:], 16.0)

            def load_xq(tb):
                xqt = xq_pool.tile([128, NE, 512], bf16, tag="xq",
                                   name=f"xq{tb}")
                nc.sync.dma_start(xqt[:].rearrange("p a b -> p (a b)"),
                                  xq_d[tb].rearrange("p a b -> p (a b)"))
                xq_tiles[tb] = xqt

            def rope_sb(dst, bsrc, bsw, tb, pool):
                """dst = rope(src) for t-block tb (dst pre-sliced, 512 cols).
                bsrc holds src straight, bsw holds src with the partition
                halves swapped — every op is same-start-partition, so all
                three run at the DVE's 2x bf16 rate."""
                tbs = bass.ts(tb, 512)
                tmp = pool.tile([128, 512], bf16, tag="tmp", name="tmp")
                tmp2 = pool.tile([128, 512], bf16, tag="tmp2", name="tmp2")
                nc.vector.tensor_mul(tmp[:], bsw[:], sns[:, tbs])
                nc.vector.tensor_mul(tmp2[:], bsrc[:], csd[:, tbs])
                nc.vector.tensor_add(dst, tmp2[:], tmp[:])

            def rope(dst, psrc, bsrc, tb, pool):
                """dst = rope(src) for t-block tb (dst pre-sliced, 512 cols).
                psrc is the [128, 512] f32 PSUM accumulator (cross-partition
                reads are only legal from PSUM), bsrc the same values already
                evicted to bf16 SBUF: the straight mul and the add run in
                bf16 at 2x on the DVE."""
                tbs = bass.ts(tb, 512)
                tmp = pool.tile([128, 512], bf16, tag="tmp", name="tmp")
                tmp2 = pool.tile([128, 512], bf16, tag="tmp2", name="tmp2")
                nc.vector.tensor_mul(tmp[0:64, :], psrc[64:128, :], sns[0:64, tbs])
                nc.vector.tensor_mul(tmp[64:128, :], psrc[0:64, :], sns[64:128, tbs])
                nc.vector.tensor_mul(tmp2[:], bsrc[:], csd[:, tbs])
                nc.vector.tensor_add(dst, tmp2[:], tmp[:])

            # ---------------- Phase 1: K + V projection ----------------
            with nc.named_scope("proj_kv"), \
                 tc.tile_pool(name="wkv", bufs=1) as wkv_pool, \
                 tc.tile_pool(name="xkv", bufs=2) as xkv_pool, \
                 tc.tile_pool(name="ksv", bufs=2) as ksv_pool, \
                 tc.tile_pool(name="kvps", bufs=1, space="PSUM") as kvps_pool, \
                 tc.tile_pool(name="rope_kv", bufs=2) as rkv_pool:
                wk = wkv_pool.tile([128, NE, ND], bf16, tag="wk")
                wv = wkv_pool.tile([128, NE, ND], bf16, tag="wv")

                def fl(ap):
                    return ap.rearrange("p a b -> p (a b)")

                # Start-of-kernel DMA is the critical path and the first
                # ~45us are HBM-bandwidth-bound: wk/wv interleaved on one
                # queue in ascending chunk sizes (arrival order = the et
                # loop's consumption order); everything not needed early
                # (wq, wo, xq) is deferred so it doesn't steal bandwidth.
                # >=2KB per-partition runs: 1KB descriptors cap the DMA
                # engines at ~40% of peak
                for a, b in ((0, 2), (2, 4), (4, 7), (7, 10), (10, 16)):
                    nc.gpsimd.dma_start(fl(wk[:, a:b, :]), fl(wk_d[:, a:b, :]))
                for a, b in ((0, 8), (8, 16)):
                    nc.gpsimd.dma_start(fl(wv[:, a:b, :]), fl(wv_d[:, a:b, :]))
                # tri + rope tables ride the otherwise idle scalar queue so
                # the sync queue goes straight from xk0 to xk1
                nc.scalar.dma_start(tri[:], tri_d[:])
                nc.scalar.dma_start(csd[:, 0:512], csd_d[:, 0:512])
                nc.scalar.dma_start(sns[:, 0:512], sns_d[:, 0:512])

                for tb in range(NB):
                    xk = xkv_pool.tile([128, NE, 512], bf16, tag="xk",
                                       name=f"xk{tb}")
                    if tb == 0:
                        for a, b in ((0, 2), (2, 4), (4, 7), (7, 10),
                                     (10, 16)):
                            nc.sync.dma_start(fl(xk[:, a:b, :]),
                                              fl(xkv_d[tb][:, a:b, :]))
                    else:
                        # split so the tb chains can start on the first
                        # chunk instead of waiting for the whole 2MB
                        nc.sync.dma_start(fl(xk[:, 0:8, :]),
                                          fl(xkv_d[tb][:, 0:8, :]))
                        nc.sync.dma_start(fl(xk[:, 8:16, :]),
                                          fl(xkv_d[tb][:, 8:16, :]))
                    if tb == 1:
                        nc.scalar.dma_start(csd[:, 512:], csd_d[:, 512:])
                        nc.scalar.dma_start(sns[:, 512:], sns_d[:, 512:])
                    if tb == 2:
                        nc.gpsimd.dma_start(fl(wq[:]), fl(wq_d[:]))
                    if tb == 3:
                        nc.gpsimd.dma_start(fl(wo[:]), fl(wo_d[:]))
                    if tb == NB - 1:
                        # Dispatch the first two Q blocks behind the last xkv
                        # block on the sync queue so Q projection never waits.
                        load_xq(0)
                        load_xq(1)
                    # All K chains of this t-block run BEFORE the V chains:
                    # the first ~25us then depend only on wk + xk0 (4MB)
                    # instead of wk + wv + xk0 (6MB) — the supply-bound head
                    # feeds the PE without stalling, and wv arrives while
                    # the K chains execute.
                    # single 4-wide K pass: the et sweep then takes 13.6us
                    # (not 6.8), matching the supply-bound DMA arrival rate
                    # of wk+xk at the kernel head
                    kps = [kvps_pool.tile([128, 512], f32, tag=f"kps{i}",
                                          name=f"kps{tb}{i}")
                           for i in range(4)]
                    if tb == 0:
                        # Warm-up: the PE waits ~4us here for the first
                        # weight/x chunks. Chew dummy matmuls on the ones
                        # tile meanwhile so the HAM clock-gate is at
                        # 2.4GHz when the real chains start.
                        for _ in range(52):
                            nc.tensor.matmul(kps[0][:, 0:128], ones[:],
                                             ones[:], start=True,
                                             stop=True)
                    for et in range(NE):
                        for i in range(4):
                            nc.tensor.matmul(
                                kps[i][:], wk[:, et, bass.ts(i, 128)],
                                xk[:, et, :], start=(et == 0),
                                stop=(et == NE - 1))
                    for i in range(4):
                        # ACT is idle in phase 1: it evicts k to bf16
                        # twice — straight and partition-swapped — so the
                        # rope runs as 3 all-bf16 DVE ops (2x mode) and
                        # the PSUM bank frees right after the ACT copies
                        ksv = ksv_pool.tile([128, 512], bf16, tag="ksv",
                                            name=f"ksv{tb}{i}")
                        ksw = ksv_pool.tile([128, 512], bf16, tag="ksw",
                                            name=f"ksw{tb}{i}")
                        nc.scalar.copy(ksv[:], kps[i][:])
                        nc.scalar.copy(ksw[0:64, :], kps[i][64:128, :])
                        nc.scalar.copy(ksw[64:128, :], kps[i][0:64, :])
                        rope_sb(kT[i][:, bass.ts(tb, 512)],
                                ksv[:], ksw[:], tb, rkv_pool)
                    vps = [kvps_pool.tile([128, 512], f32, tag=f"vps{i}",
                                          name=f"vps{tb}{i}")
                           for i in range(4)]
                    if tb == NB - 1:
                        # final pass: stagger per chain so only one vG copy
                        # holds the PSUM pool after the last matmul and
                        # phase 2's first Q chain starts sooner
                        for i in range(4):
                            for et in range(NE):
                                nc.tensor.matmul(
                                    vps[i][:], xk[:, et, bass.ts(i, 128)],
                                    wv[:, et, :], start=(et == 0),
                                    stop=(et == NE - 1))
                            nc.scalar.copy(vG[tb][:, i, :], vps[i][:])
                    else:
                        for et in range(NE):
                            for i in range(4):
                                nc.tensor.matmul(
                                    vps[i][:], xk[:, et, bass.ts(i, 128)],
                                    wv[:, et, :], start=(et == 0),
                                    stop=(et == NE - 1))
                        for i in range(4):
                            nc.scalar.copy(vG[tb][:, i, :], vps[i][:])

            # ------- Phase 2+3+4: Q projection + attention + out-proj -------
            with nc.named_scope("q_attn"), \
                 tc.tile_pool(name="qat", bufs=1) as qat_pool, \
                 tc.tile_pool(name="sps", bufs=3, space="PSUM") as sps_pool, \
                 tc.tile_pool(name="dps", bufs=1, space="PSUM") as dps_pool, \
                 tc.tile_pool(name="ups", bufs=1, space="PSUM") as ups_pool, \
                 tc.tile_pool(name="rope_q", bufs=2) as rq_pool, \
                 tc.tile_pool(name="qsv", bufs=2) as qsv_pool, \
                 tc.tile_pool(name="ob", bufs=2) as ob_pool, \
                 tc.tile_pool(name="rcp", bufs=1) as rcp_pool:
                # only the current t-block's queries are ever live
                qT = [qat_pool.tile([128, 512], bf16, tag=f"qT{h}",
                                    name=f"qT{h}") for h in range(HL)]
                # three eG sets, rotating h%3: lets QK of 3 heads run ahead
                # of the first den/AV pass without WAR serialization
                eG = [[qat_pool.tile([128, 2048], bf16, tag=f"eG{p}{g}",
                                     name=f"eG{p}{g}") for g in range(4)]
                      for p in range(3)]
                # fp8 shadow of the non-diagonal eG pairs (at most 6 per
                # head), only for the DoubleRow denominator matmuls (sum of
                # positives: the fp8 quantization noise averages out)
                eG8 = [[qat_pool.tile([128, 1024], f8, tag=f"eH{p}{j}",
                                      name=f"eH{p}{j}") for j in range(6)]
                       for p in range(3)]

                def q_mm_chain(tb, hp):
                    """Projection matmuls for heads (2hp, 2hp+1), t-block
                    tb. Returns the f32 PSUM accumulator."""
                    qps = sps_pool.tile([128, 2, 512], f32, tag="sp",
                                        name=f"qps{tb}{hp}")
                    xqt = xq_tiles[tb]
                    for et in range(NE):
                        for i in range(2):
                            h = 2 * hp + i
                            nc.tensor.matmul(
                                qps[:, i], wq[:, et, bass.ts(h, 128)],
                                xqt[:, et, :], start=(et == 0),
                                stop=(et == NE - 1))
                    return qps

                def q_evict(tb, hp, qps):
                    qsv = qsv_pool.tile([128, 2, 512], bf16, tag="qsv",
                                        name=f"qsv{tb}{hp}")
                    # evict on DVE: a copy on the in-order ACT queue would
                    # delay an exp that gates a den chain
                    nc.vector.tensor_copy(
                        qsv[:].rearrange("p a b -> p (a b)"),
                        qps[:].rearrange("p a b -> p (a b)"))
                    return qps, qsv

                def q_mm(tb, hp):
                    qps = q_mm_chain(tb, hp)
                    return q_evict(tb, hp, qps)

                def q_rope(tb, hp, qq):
                    qps, qsv = qq
                    for i in range(2):
                        rope(qT[2 * hp + i][:], qps[:, i], qsv[:, i, :], tb,
                             rq_pool)

                def attn_qk(tb, h):
                    """Scores + exp for head h of t-block tb. Diagonal pairs
                    first: their eG triangle-zero (DVE) gates the den chain,
                    so giving the in-order DVE stream maximal slack hides
                    rope/recip backlog; den/AV accumulate them first
                    symmetrically."""
                    nsi = 4 * (tb + 1)
                    eset = eG[h % 3]
                    porder = list(range(2 * tb, nsi // 2)) + list(range(2 * tb))
                    for j in porder:
                        sp = sps_pool.tile([128, 2, 512], f32, tag="sp",
                                           name=f"sp{tb}{h}{j}")
                        for p2 in range(2):
                            si = 2 * j + p2
                            v = si - 4 * tb
                            # diagonal slice v: cols [0, 128v) are fully below
                            # the causal boundary — skip them in QK (and in
                            # den/AV below); exp still covers them but writes
                            # garbage that is never read.
                            off = 128 * v if v > 0 else 0
                            nc.tensor.matmul(
                                sp[:, p2, off:512], kT[h][:, bass.ts(si, 128)],
                                qT[h][:, bass.ds(off, 512 - off)],
                                start=True, stop=True)
                        if j >= 2 * tb:
                            # diagonal pair: exp ONLY the written region of
                            # each slice. Reading the unwritten [0,128v)
                            # PSUM prefix creates a false RAW dependency on
                            # that bank's PREVIOUS writer (often a fresh
                            # q_mm chain) and stalled den by ~1us per block.
                            for p2 in range(2):
                                off = 128 * (2 * (j - 2 * tb) + p2)
                                nc.scalar.activation(
                                    eset[j // 2][:, bass.ds(
                                        1024 * (j % 2) + 512 * p2 + off,
                                        512 - off)],
                                    sp[:, p2, off:512], Exp)
                        else:
                            nc.scalar.activation(
                                eset[j // 2][:, bass.ts(j % 2, 1024)],
                                sp[:].rearrange("p a b -> p (a b)"), Exp)
                        for p2 in range(2):
                            si = 2 * j + p2
                            v = si - 4 * tb
                            if v >= 0:
                                # causal boundary tile: zero the upper
                                # triangle of eG post-exp (bf16, off the
                                # pre-exp critical path, on the otherwise
                                # idle GpSimd so it never queues)
                                reg = eset[j // 2][:, bass.ds(
                                    1024 * (j % 2) + 512 * p2 + 128 * v, 128)]
                                nc.gpsimd.tensor_mul(reg, reg, tri[:])
                        if 2 * j + 1 < 4 * tb:
                            # non-diagonal pair: fp8 shadow for the
                            # DoubleRow denominator, scaled 1/16 to dodge
                            # the fp8e4 240-max overflow (ones8 holds 16)
                            nc.vector.tensor_scalar_mul(
                                eG8[h % 3][j][:],
                                eset[j // 2][:, bass.ts(j % 2, 1024)],
                                0.0625)

                def attn_dv(tb, h):
                    """Denominator + A@V + normalization for head h."""
                    nsi = 4 * (tb + 1)
                    eset = eG[h % 3]
                    sorder = list(range(4 * tb, nsi)) + list(range(4 * tb))

                    def e_off(si):
                        # skip the fully-masked prefix of diagonal slices;
                        # the first chain matmul (si=4tb, off=0) covers every
                        # column, so start=True initializes the whole bank
                        v = si - 4 * tb
                        return 128 * v if v > 0 else 0

                    den = dps_pool.tile([128, 512], f32, tag="den",
                                        name=f"den{tb}{h}")
                    # diagonal slices in bf16 (off-trimmed), the 4*tb
                    # non-diagonal slices as fp8 DoubleRow pairs (2 slices
                    # contracted per instruction). NOTE: a reordering that
                    # puts the tri-gated boundary tiles last was tried and
                    # measured ~6us SLOWER (extra instructions + a bf16
                    # pair-0, since DoubleRow + start=True mis-initializes
                    # PSUM) — keep the simple order.
                    for i, si in enumerate(range(4 * tb, nsi)):
                        off = e_off(si)
                        nc.tensor.matmul(
                            den[:, off:512], ones[:],
                            eset[si // 4][:, bass.ds(512 * (si % 4) + off,
                                                     512 - off)],
                            start=(i == 0), stop=(i == 3 and tb == 0))
                    for jp in range(2 * tb):
                        nc.tensor.matmul(
                            den[:], ones8[:],
                            eG8[h % 3][jp][:]
                            .rearrange("p (a b) -> p a b", a=2),
                            start=False, stop=(jp == 2 * tb - 1),
                            perf_mode=DR)
                    rec = rcp_pool.tile([128, 512], f32, tag="rec", name="rec")
                    nc.vector.reciprocal_approx_fast(rec[:], den[:])
                    up = ups_pool.tile([128, 512], f32, tag="up",
                                       name=f"up{tb}{h}")
                    for i, si in enumerate(sorder):
                        off = e_off(si)
                        nc.tensor.matmul(
                            up[:, off:512], vG[si // 4][:, si % 4, bass.ts(h, 128)],
                            eset[si // 4][:, bass.ds(512 * (si % 4) + off,
                                                     512 - off)],
                            start=(i == 0), stop=(i == nsi - 1))
                    nc.vector.tensor_mul(uT[h][:, bass.ts(tb, 512)], up[:],
                                         rec[:])

                def op_chain(tb, tt, ecp, split=False, act_evict=False):
                    """Out-projection for row tile tt, E columns
                    [1024*ecp, 1024*(ecp+1)): contract 4 heads into a
                    [128,2,512] psum pair, evict, store."""
                    ops = sps_pool.tile([128, 2, 512], f32, tag="sp",
                                        name=f"ops{tt}{ecp}")
                    for h in range(HL):
                        for i in range(2):
                            nc.tensor.matmul(
                                ops[:, i], uT[h][:, bass.ts(tt, 128)],
                                wo[:, h, bass.ds(1024 * ecp + 512 * i, 512)],
                                start=(h == 0), stop=(h == HL - 1))
                    ob = ob_pool.tile([128, 1024], bf16, tag="ob",
                                      name=f"ob{tt}{ecp}")
                    if split:
                        # kernel tail (no exps left): drain in 512-col pieces
                        # across both evict engines and both store queues
                        nc.vector.tensor_copy(ob[:, 0:512], ops[:, 0])
                        nc.sync.dma_start(
                            out_d[tt][:, bass.ds(1024 * ecp, 512)],
                            ob[:, 0:512])
                        nc.scalar.copy(ob[:, 512:1024], ops[:, 1])
                        nc.scalar.dma_start(
                            out_d[tt][:, bass.ds(1024 * ecp + 512, 512)],
                            ob[:, 512:1024])
                        return
                    # evict on DVE only, store on the sync queue only: both
                    # a copy and a store-dispatch on the in-order ACT queue
                    # would delay an exp that gates a den chain. In the tail
                    # (act_evict) the exps are done and ACT helps out.
                    if act_evict:
                        nc.scalar.copy(ob[:],
                                       ops[:].rearrange("p a b -> p (a b)"))
                    else:
                        nc.vector.tensor_copy(
                            ob[:], ops[:].rearrange("p a b -> p (a b)"))
                    nc.sync.dma_start(out_d[tt][:, bass.ds(1024 * ecp, 1024)],
                                      ob[:])

                def attn_block(tb, qmid=None, qnext=None, fillers=()):
                    """Full attention t-block. qmid (this block's second Q
                    chain) runs right after qk0 so the exp stream gets ~3.4us
                    of PE-covered catch-up before den(0) needs it; qnext (the
                    NEXT block's first Q chain) covers the den(3) tail.
                    Out-projection chains of the previous t-block (fillers)
                    are woven in as PE work with no ACT/DVE dependencies."""
                    fill = iter(fillers)

                    def F(n):
                        for _ in range(n):
                            t = next(fill, None)
                            if t is not None:
                                t()

                    early_q = qnext is not None and not fillers
                    attn_qk(tb, 0)
                    attn_qk(tb, 1)
                    F(1)
                    attn_qk(tb, 2)
                    F(2)
                    if early_q:
                        # no filler chains exist (block 0): the next Q chain
                        # covers the exp catch-up before den(0) instead
                        qnext()
                    attn_dv(tb, 0)
                    attn_qk(tb, 3)
                    F(1)
                    attn_dv(tb, 1)
                    F(1)
                    attn_dv(tb, 2)
                    if qnext is not None and not early_q:
                        qnext()
                    attn_dv(tb, 3)
                    F(99)

                qq = q_mm(0, 0)
                q_rope(0, 0, qq)
                fillers = []
                for tb in range(NB):
                    if tb + 2 < NB:
                        load_xq(tb + 2)
                    # second Q chain of this block runs before the block: its
                    # 3.4us of PE covers the first Q pair's rope latency
                    qq1 = q_mm(tb, 1)
                    q_rope(tb, 1, qq1)

                    holder = {}

                    def qnext(tb=tb, holder=holder):
                        # next block's first Q chain covers the den(3) tail
                        # on the PE; the DVE-side evict+ropes are deferred
                        # until after the block so they don't delay norm(3)
                        # (which gates the next block's out-proj fillers)
                        holder["qps"] = q_mm_chain(tb + 1, 0)

                    attn_block(tb, qnext=qnext if tb + 1 < NB else None,
                               fillers=fillers)
                    if tb + 1 < NB:
                        qq0 = q_evict(tb + 1, 0, holder["qps"])
                        q_rope(tb + 1, 0, qq0)
                    fillers = [
                        (lambda tt=tt, ecp=ecp: op_chain(tb, tt, ecp))
                        for tt in range(4 * tb, 4 * tb + 4)
                        for ecp in range(2)]
                # tail: the last t-block's own out-projection, evictions
                # alternating across ACT/DVE (no exps left to delay)
                for tt in range(4 * (NB - 1), 4 * NB):
                    for ecp in range(2):
                        op_chain(NB - 1, tt, ecp,
                                 split=(tt == 4 * NB - 1 and ecp == 1),
                                 act_evict=(ecp == 0))

    nc.compile()
    return nc


def _get_module():
    if "nc" not in _NC_CACHE:
        _NC_CACHE["nc"] = _build_module()
    return _NC_CACHE["nc"]


def _host_prep(inputs_q, inputs_kv, positions, Wq, Wk, Wv, Wo):
    """Build the 8 per-core input maps (device-packed layouts, bf16)."""
    import ml_dtypes
    bf16 = ml_dtypes.bfloat16
    perm = np.concatenate([np.arange(0, D, 2), np.arange(1, D, 2)])  # de-interleave
    scale = np.float32(1.0 / np.sqrt(D))
    half = D // 2
    timescale = 10000.0 ** (2.0 * np.arange(half, dtype=np.float64) / D)
    # 0/1 causal triangle for the boundary tile: keep where col >= row
    s_i = np.arange(128)[:, None]
    tri = (np.arange(128)[None, :] >= s_i).astype(bf16)

    def pack_x(xT):
        # [E, S] f32 -> [NB, 128, NE, 512]: x_p[tb, p, et, t] = xT[128 et + p, 512 tb + t]
        return np.ascontiguousarray(
            xT.reshape(NE, 128, NB, 512).transpose(2, 1, 0, 3).astype(bf16))

    def pack_w(w):
        # [E, ND] -> [128, NE, ND]: w_p[p, et, n] = w[128 et + p, n]
        return np.ascontiguousarray(
            w.reshape(NE, 128, ND).transpose(1, 0, 2).astype(bf16))

    in_maps = []
    for c in range(8):
        b = c // 4
        h0 = (c % 4) * HL
        angle = positions[b].astype(np.float64)[None, :] / timescale[:, None]  # [64,S]
        cs = np.cos(angle)
        sn = np.sin(angle)
        csd = np.concatenate([cs, cs], axis=0).astype(bf16)   # [128, S]
        sns = np.concatenate([-sn, sn], axis=0).astype(bf16)  # [128, S]
        wq = (Wq[:, h0:h0 + HL, :][:, :, perm] * scale).reshape(E, ND)
        wk = Wk[:, h0:h0 + HL, :][:, :, perm].reshape(E, ND)
        wv = Wv[:, h0:h0 + HL, :].reshape(E, ND)
        wo = Wo[h0:h0 + HL]                                   # [HL, D, E]
        in_maps.append({
            "xq_p": pack_x(np.asarray(inputs_q[b]).T),
            "xkv_p": pack_x(np.asarray(inputs_kv[b]).T),
            "wq_p": pack_w(np.asarray(wq, dtype=np.float32)),
            "wk_p": pack_w(np.asarray(wk, dtype=np.float32)),
            "wv_p": pack_w(np.asarray(wv, dtype=np.float32)),
            "wo_p": np.ascontiguousarray(
                np.asarray(wo, dtype=np.float32).transpose(1, 0, 2).astype(bf16)),
            "csd": csd, "sns": sns, "tri": tri,
        })
    return in_maps


def kernel(inputs_q, inputs_kv, positions, Wq, Wk, Wv, Wo, _trace=False,
           _trace_kwargs=None):
    from concourse import bass_utils

    nc = _get_module()
    in_maps = _host_prep(inputs_q, inputs_kv, positions, Wq, Wk, Wv, Wo)
    res = bass_utils.run_bass_kernel_spmd(
        nc, in_maps, core_ids=list(range(8)), trace=_trace,
        **(_trace_kwargs or {}))
    if _trace:
        _NC_CACHE["last_results"] = res
    parts = [np.asarray(res.results[c]["out"], dtype=np.float32).reshape(S, E)
             for c in range(8)]
    out0 = parts[0] + parts[1] + parts[2] + parts[3]
    out1 = parts[4] + parts[5] + parts[6] + parts[7]
    return np.stack([out0, out1]).astype(np.float32)
